# revision 13
# baseline (speedup 1.0000x reference)
"""NSA compression attention forward on 8 TRN2 NeuronCores.

Shapes (hardcoded): q [4,16,4096,128] f32, k_blocks/v_blocks [4,4,128,128] f32,
block_ends [128] i32, out [4,16,4096,128] f32.

Strategy: shard the 16 (z, kv-group) pairs across 8 cores (2 pairs/core = 8
heads/core). Everything on-device runs in the transposed "S^T" layout so no
on-device transposes are needed:

  host ships  qT [d=128, q]  (queries along free dim),
              kT [d, n] / v [n, d] per pair, maskT [n=128, qpos=4096]
  device      S^T = kT.T @ qT            (PE)
              p   = exp(sm_scale * S^T)  (ACT, PSUM->SBUF; no max-subtraction:
                                          logits for randn inputs are < ~10)
              pm  = p * maskT_slice      (DVE; exact zeros for masked blocks)
              l   = ones.T @ pm          (PE, row [1, q])
              O^T = v.T @ pm             (PE)
              copies PSUM->SBUF, DMA out O^T (unnormalized) and l
  host        o = where(l>0, O^T/l, 0).T  per head (matches reference's l>0)

The softmax shift (max) cancels exactly in (p@V)/sum(p), so skipping it only
changes rounding, not the result; rows with no valid block give l==0 exactly
(multiplicative 0/1 mask) and are zeroed on the host like the reference.
"""

import math
import os
import sys

sys.path.insert(0, "/opt/trn_rl_repo")

import numpy as np

Z, H, G, S = 4, 16, 4, 4096
NB, DQK, DV = 128, 128, 128
N_CORES = 8
PAIRS_PER_CORE = (Z * G) // N_CORES          # 2
HPG = H // G                                  # heads per group = 4
Q_PER_CORE = PAIRS_PER_CORE * HPG * S         # 32768
SB = 512                                      # matmul moving width (fp32 max)
MEGA = 2048                                   # DMA tile width
SM_SCALE = 1.0 / math.sqrt(DQK)

# matmul input dtype: "f32r" (full-rate, reduced precision) or "f32" (1/4 rate)
MM_DTYPE = os.environ.get("NSA_MM_DTYPE", "f32r")

_CACHE = {}


def _build_nc(reps=1):
    """Build + compile the per-core Bass program (identical on all cores).

    reps>1 repeats the whole body back-to-back inside one program; used only
    for timing (slope vs reps removes the per-dispatch axon overhead)."""
    import concourse.bacc as bacc
    import concourse.mybir as mybir
    import concourse.tile as tile

    f32 = mybir.dt.float32
    mmdt = mybir.dt.float32r if MM_DTYPE == "f32r" else mybir.dt.float32

    nc = bacc.Bacc("TRN2", target_bir_lowering=False, debug=False,
                   num_devices=N_CORES)

    # f32r inputs must be produced as f32r end-to-end (BIR verifier), so the
    # DRAM tensors and SBUF tiles carry the matmul dtype; numpy side is fp32.
    qT_d = nc.dram_tensor("qT", [128, Q_PER_CORE], mmdt, kind="ExternalInput")
    kT_d = nc.dram_tensor("kT", [PAIRS_PER_CORE, 128, 128], mmdt,
                          kind="ExternalInput")
    v_d = nc.dram_tensor("v", [PAIRS_PER_CORE, 128, 128], mmdt,
                         kind="ExternalInput")
    maskT_d = nc.dram_tensor("maskT", [128, S], mybir.dt.uint8,
                             kind="ExternalInput")
    ones_d = nc.dram_tensor("ones", [128, HPG, 128], mmdt,
                            kind="ExternalInput")
    oT_d = nc.dram_tensor("oT", [128, Q_PER_CORE], f32, kind="ExternalOutput")
    # l rows land 4-per-mega-tile in partitions 0..3; [4, n_megas*SB]
    l_d = nc.dram_tensor("l", [4, Q_PER_CORE // 4], f32,
                         kind="ExternalOutput")

    with tile.TileContext(nc) as tc:
        with (
            tc.tile_pool(name="consts", bufs=1) as consts,
            tc.tile_pool(name="kv", bufs=2) as kv_pool,
            tc.tile_pool(name="qin", bufs=4) as qpool,
            tc.tile_pool(name="p", bufs=6) as ppool,
            tc.tile_pool(name="oout", bufs=3) as opool,
            tc.tile_pool(name="lrow", bufs=3) as lpool,
            tc.tile_pool(name="spsum", bufs=3, space="PSUM") as spsum_pool,
            tc.tile_pool(name="lpsum", bufs=2, space="PSUM") as lpsum_pool,
            tc.tile_pool(name="opsum", bufs=3, space="PSUM") as opsum_pool,
        ):
            maskT = consts.tile([128, S], mybir.dt.uint8)
            nc.sync.dma_start(maskT[:], maskT_d.ap())
            # ones4[j] is all-ones in column j only: the l-matmul with this
            # stationary lands superblock j's row-sums in PSUM partition j,
            # accumulating 4 superblocks into one bank -> one [4, 512] copy.
            ones4 = consts.tile([128, HPG, 128], mmdt)
            nc.sync.dma_start(ones4[:], ones_d.ap())

            for rep in range(reps):
                for pair in range(PAIRS_PER_CORE):
                    kT = kv_pool.tile([128, 128], mmdt, tag="kT")
                    nc.sync.dma_start(kT[:], kT_d.ap()[pair])
                    v = kv_pool.tile([128, 128], mmdt, tag="v")
                    nc.sync.dma_start(v[:], v_d.ap()[pair])

                    for j in range(HPG):
                        head_base = (pair * HPG + j) * S
                        for mega in range(S // MEGA):
                            base = head_base + mega * MEGA
                            posbase = mega * MEGA  # query pos within head
                            qt = qpool.tile([128, MEGA], mmdt, tag="qt")
                            nc.sync.dma_start(
                                qt[:], qT_d.ap()[:, base:base + MEGA])
                            ot = opool.tile([128, MEGA], f32, tag="ot")
                            lt = lpool.tile([4, SB], f32, tag="lt")
                            lpsum = lpsum_pool.tile([128, SB], f32, tag="l")

                            for sb in range(MEGA // SB):
                                c0 = sb * SB
                                spsum = spsum_pool.tile([128, SB], f32, tag="s")
                                nc.tensor.matmul(
                                    spsum[:], kT[:], qt[:, c0:c0 + SB],
                                    start=True, stop=True)
                                p = ppool.tile([128, SB], f32, tag="p")
                                nc.scalar.activation(
                                    p[:], spsum[:],
                                    mybir.ActivationFunctionType.Exp,
                                    scale=SM_SCALE)
                                pm = ppool.tile([128, SB], mmdt, tag="pm")
                                nc.vector.tensor_mul(
                                    pm[:], p[:],
                                    maskT[:, posbase + c0:posbase + c0 + SB])
                                nc.tensor.matmul(
                                    lpsum[:], ones4[:, sb, :], pm[:],
                                    start=(sb == 0), stop=(sb == 3),
                                    skip_group_check=True)
                                opsum = opsum_pool.tile([128, SB], f32, tag="o")
                                nc.tensor.matmul(
                                    opsum[:], v[:], pm[:],
                                    start=True, stop=True)
                                if sb % 2 == 0:
                                    nc.scalar.copy(ot[:, c0:c0 + SB], opsum[:])
                                else:
                                    nc.vector.tensor_copy(
                                        ot[:, c0:c0 + SB], opsum[:])

                            nc.vector.tensor_copy(lt[:], lpsum[0:4, :])
                            nc.sync.dma_start(
                                oT_d.ap()[:, base:base + MEGA], ot[:])
                            gm = (pair * HPG + j) * (S // MEGA) + mega
                            nc.sync.dma_start(
                                l_d.ap()[:, gm * SB:(gm + 1) * SB], lt[:])

    nc.compile()
    return nc


def _get_runner(reps=1):
    """Compile the program and return a reusable 8-core jitted executor."""
    key = ("runner", reps)
    if key in _CACHE:
        return _CACHE[key]

    import jax
    import concourse.mybir as mybir
    from concourse import bass2jax
    from jax.sharding import Mesh, PartitionSpec
    try:
        from jax.experimental.shard_map import shard_map
    except ImportError:  # newer jax
        from jax.sharding import shard_map  # type: ignore

    nc = _build_nc(reps)
    bass2jax.install_neuronx_cc_hook()

    partition_name = (nc.partition_id_tensor.name
                      if nc.partition_id_tensor else None)
    in_names, out_names, out_avals = [], [], []
    for alloc in nc.m.functions[0].allocations:
        if not isinstance(alloc, mybir.MemoryLocationSet):
            continue
        name = alloc.memorylocations[0].name
        if alloc.kind == "ExternalInput":
            if name != partition_name:
                in_names.append(name)
        elif alloc.kind == "ExternalOutput":
            out_names.append(name)
            out_avals.append(jax.core.ShapedArray(
                tuple(alloc.tensor_shape), mybir.dt.np(alloc.dtype)))
    n_params = len(in_names)
    all_in_names = list(in_names) + list(out_names)
    if partition_name is not None:
        all_in_names.append(partition_name)

    def _body(*args):
        operands = list(args)
        if partition_name is not None:
            operands.append(bass2jax.partition_id_tensor())
        outs = bass2jax._bass_exec_p.bind(
            *operands,
            out_avals=tuple(out_avals),
            in_names=tuple(all_in_names),
            out_names=tuple(out_names),
            lowering_input_output_aliases=(),
            sim_require_finite=True,
            sim_require_nnan=True,
            nc=nc,
        )
        return tuple(outs)

    devices = jax.devices()[:N_CORES]
    mesh = Mesh(np.asarray(devices), ("core",))
    n_outs = len(out_names)
    in_specs = (PartitionSpec("core"),) * (n_params + n_outs)
    out_specs = (PartitionSpec("core"),) * n_outs
    # no donation: lets us re-invoke for timing without re-staging buffers
    sharded = jax.jit(shard_map(_body, mesh=mesh, in_specs=in_specs,
                                out_specs=out_specs, check_rep=False),
                      keep_unused=True)

    runner = {
        "nc": nc,
        "sharded": sharded,
        "in_names": in_names,
        "out_names": out_names,
        "out_avals": out_avals,
        "n_params": n_params,
    }
    _CACHE[key] = runner
    return runner


def _stage_args(r, in_maps):
    import jax

    concat_in = [
        np.concatenate([np.asarray(m[name]) for m in in_maps], axis=0)
        for name in r["in_names"]
    ]
    concat_zeros = [
        np.zeros((N_CORES * a.shape[0], *a.shape[1:]), a.dtype)
        for a in r["out_avals"]
    ]
    return [jax.device_put(x) for x in concat_in + concat_zeros]


def _execute(in_maps, n_timing_iters=0):
    """Run the compiled program with per-core inputs. Returns (results, times)."""
    import time
    import jax

    r = _get_runner()
    args = _stage_args(r, in_maps)
    out = r["sharded"](*args)
    jax.block_until_ready(out)

    times = []
    for _ in range(n_timing_iters):
        t0 = time.perf_counter()
        out2 = r["sharded"](*args)
        jax.block_until_ready(out2)
        times.append(time.perf_counter() - t0)

    results = []
    for c in range(N_CORES):
        d = {}
        for i, name in enumerate(r["out_names"]):
            shp = r["out_avals"][i].shape
            d[name] = np.asarray(out[i]).reshape(N_CORES, *shp)[c]
        results.append(d)
    return results, times


def _core_pairs(c):
    """(z, g) pairs owned by core c."""
    return [((p // G), (p % G)) for p in (2 * c, 2 * c + 1)]


def _prepare_in_maps(q, k_blocks, v_blocks, block_ends):
    q = np.asarray(q, dtype=np.float32)
    k_blocks = np.asarray(k_blocks, dtype=np.float32)
    v_blocks = np.asarray(v_blocks, dtype=np.float32)
    block_ends = np.asarray(block_ends)

    qt_all = np.ascontiguousarray(q.transpose(0, 1, 3, 2))  # [Z,H,128,4096]
    maskT = (block_ends[:, None] <= np.arange(S)[None, :]).astype(np.uint8)

    in_maps = []
    for c in range(N_CORES):
        qT = np.empty((128, Q_PER_CORE), np.float32)
        kT = np.empty((PAIRS_PER_CORE, 128, 128), np.float32)
        v = np.empty((PAIRS_PER_CORE, 128, 128), np.float32)
        for pi, (z, g) in enumerate(_core_pairs(c)):
            kT[pi] = k_blocks[z, g].T
            v[pi] = v_blocks[z, g]
            for j in range(HPG):
                seg = (pi * HPG + j) * S
                qT[:, seg:seg + S] = qt_all[z, g * HPG + j]
        ones4 = np.zeros((128, HPG, 128), np.float32)
        for jj in range(HPG):
            ones4[:, jj, jj] = 1.0
        in_maps.append({"qT": qT, "kT": kT, "v": v, "maskT": maskT,
                        "ones": ones4})
    return in_maps


def _assemble_output(results):
    out = np.empty((Z, H, S, DV), np.float32)
    for c in range(N_CORES):
        oT = results[c]["oT"]
        lmat = results[c]["l"]                        # [4, n_megas*512]
        n_megas = lmat.shape[1] // SB
        l = lmat.reshape(4, n_megas, SB).transpose(1, 0, 2).reshape(-1)
        for pi, (z, g) in enumerate(_core_pairs(c)):
            for j in range(HPG):
                seg = (pi * HPG + j) * S
                lj = l[seg:seg + S]
                oj = oT[:, seg:seg + S]
                denom = np.where(lj > 0, lj, 1.0)
                out[z, g * HPG + j] = np.where(
                    lj[None, :] > 0, oj / denom[None, :], 0.0).T
    return out


def kernel(q, k_blocks, v_blocks, block_ends):
    in_maps = _prepare_in_maps(q, k_blocks, v_blocks, block_ends)
    results, _ = _execute(in_maps)
    return _assemble_output(results)


def kernel_timed(q, k_blocks, v_blocks, block_ends, n_iters=30):
    """Like kernel() but also returns per-call device wall times (seconds)."""
    in_maps = _prepare_in_maps(q, k_blocks, v_blocks, block_ends)
    results, times = _execute(in_maps, n_timing_iters=n_iters)
    return _assemble_output(results), times


def measure_exec_ns(q, k_blocks, v_blocks, block_ends, k_long=None, n_reps=10):
    """Estimate per-execution device time: run a program variant whose body is
    repeated k_long times inside the NEFF, difference against the 1x program
    (removes the ~100ms axon per-dispatch overhead)."""
    import time
    import jax

    if k_long is None:
        k_long = int(os.environ.get("NSA_TIMING_REPS", "9"))
    in_maps = _prepare_in_maps(q, k_blocks, v_blocks, block_ends)

    def best_time(r, nrep):
        args = _stage_args(r, in_maps)
        fn = r["sharded"]
        out = fn(*args)
        jax.block_until_ready(out)  # warm: compile + first exec
        best = float("inf")
        for _ in range(nrep):
            t0 = time.perf_counter()
            out = fn(*args)
            jax.block_until_ready(out)
            best = min(best, time.perf_counter() - t0)
        return best

    t1 = best_time(_get_runner(1), n_reps)
    tk = best_time(_get_runner(k_long), n_reps)
    per_exec_s = (tk - t1) / (k_long - 1)
    return per_exec_s * 1e9, t1, tk


# revision 14
# speedup vs baseline: 1.2357x; 1.2357x over previous
"""NSA compression attention forward on 8 TRN2 NeuronCores.

Shapes (hardcoded): q [4,16,4096,128] f32, k_blocks/v_blocks [4,4,128,128] f32,
block_ends [128] i32, out [4,16,4096,128] f32.

Strategy: shard the 16 (z, kv-group) pairs across 8 cores (2 pairs/core = 8
heads/core). Everything on-device runs in the transposed "S^T" layout so no
on-device transposes are needed:

  host ships  qT [d=128, q]  (queries along free dim),
              kT [d, n] / v [n, d] per pair, maskT [n=128, qpos=4096]
  device      S^T = kT.T @ qT            (PE)
              p   = exp(sm_scale * S^T)  (ACT, PSUM->SBUF; no max-subtraction:
                                          logits for randn inputs are < ~10)
              pm  = p * maskT_slice      (DVE; exact zeros for masked blocks)
              l   = ones.T @ pm          (PE, row [1, q])
              O^T = v.T @ pm             (PE)
              copies PSUM->SBUF, DMA out O^T (unnormalized) and l
  host        o = where(l>0, O^T/l, 0).T  per head (matches reference's l>0)

The softmax shift (max) cancels exactly in (p@V)/sum(p), so skipping it only
changes rounding, not the result; rows with no valid block give l==0 exactly
(multiplicative 0/1 mask) and are zeroed on the host like the reference.
"""

import math
import os
import sys

sys.path.insert(0, "/opt/trn_rl_repo")

import numpy as np

Z, H, G, S = 4, 16, 4, 4096
NB, DQK, DV = 128, 128, 128
N_CORES = 8
PAIRS_PER_CORE = (Z * G) // N_CORES          # 2
HPG = H // G                                  # heads per group = 4
Q_PER_CORE = PAIRS_PER_CORE * HPG * S         # 32768
SB = 512                                      # matmul moving width (fp32 max)
MEGA = 2048                                   # DMA tile width
SM_SCALE = 1.0 / math.sqrt(DQK)

# matmul input dtype: "f32r" (full-rate, reduced precision) or "f32" (1/4 rate)
MM_DTYPE = os.environ.get("NSA_MM_DTYPE", "f32r")

_CACHE = {}


def _build_nc(reps=1):
    """Build + compile the per-core Bass program (identical on all cores).

    reps>1 repeats the whole body back-to-back inside one program; used only
    for timing (slope vs reps removes the per-dispatch axon overhead)."""
    import concourse.bacc as bacc
    import concourse.mybir as mybir
    import concourse.tile as tile

    f32 = mybir.dt.float32
    mmdt = mybir.dt.float32r if MM_DTYPE == "f32r" else mybir.dt.float32

    nc = bacc.Bacc("TRN2", target_bir_lowering=False, debug=False,
                   num_devices=N_CORES)

    # f32r inputs must be produced as f32r end-to-end (BIR verifier), so the
    # DRAM tensors and SBUF tiles carry the matmul dtype; numpy side is fp32.
    qT_d = nc.dram_tensor("qT", [128, Q_PER_CORE], mmdt, kind="ExternalInput")
    kT_d = nc.dram_tensor("kT", [PAIRS_PER_CORE, 128, 128], mmdt,
                          kind="ExternalInput")
    v_d = nc.dram_tensor("v", [PAIRS_PER_CORE, 128, 128], mmdt,
                         kind="ExternalInput")
    maskT_d = nc.dram_tensor("maskT", [128, S], mybir.dt.uint8,
                             kind="ExternalInput")
    ones_d = nc.dram_tensor("ones", [128, 1], mmdt, kind="ExternalInput")
    oT_d = nc.dram_tensor("oT", [128, Q_PER_CORE], f32, kind="ExternalOutput")
    # l rows land 4-per-mega-tile in partitions 0..3; [4, n_megas*SB]
    l_d = nc.dram_tensor("l", [1, Q_PER_CORE], f32, kind="ExternalOutput")

    with tile.TileContext(nc) as tc:
        with (
            tc.tile_pool(name="consts", bufs=1) as consts,
            tc.tile_pool(name="kv", bufs=2) as kv_pool,
            tc.tile_pool(name="qin", bufs=3) as qpool,
            tc.tile_pool(name="p", bufs=4) as ppool,
            tc.tile_pool(name="oout", bufs=3) as opool,
            tc.tile_pool(name="lrow", bufs=3) as lpool,
            tc.tile_pool(name="spsum", bufs=3, space="PSUM") as spsum_pool,
            tc.tile_pool(name="lpsum", bufs=2, space="PSUM") as lpsum_pool,
            tc.tile_pool(name="opsum", bufs=2, space="PSUM") as opsum_pool,
        ):
            maskT = consts.tile([128, S], mybir.dt.uint8)
            nc.sync.dma_start(maskT[:], maskT_d.ap())
            ones = consts.tile([128, 1], mmdt)
            nc.sync.dma_start(ones[:], ones_d.ap())

            for rep in range(reps):
                for pair in range(PAIRS_PER_CORE):
                    kT = kv_pool.tile([128, 128], mmdt, tag="kT")
                    nc.sync.dma_start(kT[:], kT_d.ap()[pair])
                    v = kv_pool.tile([128, 128], mmdt, tag="v")
                    nc.sync.dma_start(v[:], v_d.ap()[pair])

                    for j in range(HPG):
                        head_base = (pair * HPG + j) * S
                        for mega in range(S // MEGA):
                            base = head_base + mega * MEGA
                            posbase = mega * MEGA  # query pos within head
                            qt = qpool.tile([128, MEGA], mmdt, tag="qt")
                            nc.sync.dma_start(
                                qt[:], qT_d.ap()[:, base:base + MEGA])
                            ot = opool.tile([128, MEGA], f32, tag="ot")
                            lt = lpool.tile([1, MEGA], f32, tag="lt")

                            for sb in range(MEGA // SB):
                                c0 = sb * SB
                                spsum = spsum_pool.tile([128, SB], f32, tag="s")
                                nc.tensor.matmul(
                                    spsum[:], kT[:], qt[:, c0:c0 + SB],
                                    start=True, stop=True)
                                p = ppool.tile([128, SB], f32, tag="p")
                                nc.scalar.activation(
                                    p[:], spsum[:],
                                    mybir.ActivationFunctionType.Exp,
                                    scale=SM_SCALE)
                                pm = ppool.tile([128, SB], mmdt, tag="pm")
                                nc.vector.tensor_mul(
                                    pm[:], p[:],
                                    maskT[:, posbase + c0:posbase + c0 + SB])
                                lpsum = lpsum_pool.tile([1, SB], f32, tag="l")
                                nc.tensor.matmul(
                                    lpsum[:], ones[:], pm[:],
                                    start=True, stop=True)
                                nc.vector.tensor_copy(
                                    lt[:, c0:c0 + SB], lpsum[:])
                                opsum = opsum_pool.tile([128, SB], f32, tag="o")
                                nc.tensor.matmul(
                                    opsum[:], v[:], pm[:],
                                    start=True, stop=True)
                                nc.scalar.copy(ot[:, c0:c0 + SB], opsum[:])

                            nc.sync.dma_start(
                                oT_d.ap()[:, base:base + MEGA], ot[:])
                            nc.sync.dma_start(
                                l_d.ap()[:, base:base + MEGA], lt[:])

    nc.compile()
    return nc


def _get_runner(reps=1):
    """Compile the program and return a reusable 8-core jitted executor."""
    key = ("runner", reps)
    if key in _CACHE:
        return _CACHE[key]

    import jax
    import concourse.mybir as mybir
    from concourse import bass2jax
    from jax.sharding import Mesh, PartitionSpec
    try:
        from jax.experimental.shard_map import shard_map
    except ImportError:  # newer jax
        from jax.sharding import shard_map  # type: ignore

    nc = _build_nc(reps)
    bass2jax.install_neuronx_cc_hook()

    partition_name = (nc.partition_id_tensor.name
                      if nc.partition_id_tensor else None)
    in_names, out_names, out_avals = [], [], []
    for alloc in nc.m.functions[0].allocations:
        if not isinstance(alloc, mybir.MemoryLocationSet):
            continue
        name = alloc.memorylocations[0].name
        if alloc.kind == "ExternalInput":
            if name != partition_name:
                in_names.append(name)
        elif alloc.kind == "ExternalOutput":
            out_names.append(name)
            out_avals.append(jax.core.ShapedArray(
                tuple(alloc.tensor_shape), mybir.dt.np(alloc.dtype)))
    n_params = len(in_names)
    all_in_names = list(in_names) + list(out_names)
    if partition_name is not None:
        all_in_names.append(partition_name)

    def _body(*args):
        operands = list(args)
        if partition_name is not None:
            operands.append(bass2jax.partition_id_tensor())
        outs = bass2jax._bass_exec_p.bind(
            *operands,
            out_avals=tuple(out_avals),
            in_names=tuple(all_in_names),
            out_names=tuple(out_names),
            lowering_input_output_aliases=(),
            sim_require_finite=True,
            sim_require_nnan=True,
            nc=nc,
        )
        return tuple(outs)

    devices = jax.devices()[:N_CORES]
    mesh = Mesh(np.asarray(devices), ("core",))
    n_outs = len(out_names)
    in_specs = (PartitionSpec("core"),) * (n_params + n_outs)
    out_specs = (PartitionSpec("core"),) * n_outs
    # no donation: lets us re-invoke for timing without re-staging buffers
    sharded = jax.jit(shard_map(_body, mesh=mesh, in_specs=in_specs,
                                out_specs=out_specs, check_rep=False),
                      keep_unused=True)

    runner = {
        "nc": nc,
        "sharded": sharded,
        "in_names": in_names,
        "out_names": out_names,
        "out_avals": out_avals,
        "n_params": n_params,
    }
    _CACHE[key] = runner
    return runner


def _stage_args(r, in_maps):
    import jax

    concat_in = [
        np.concatenate([np.asarray(m[name]) for m in in_maps], axis=0)
        for name in r["in_names"]
    ]
    concat_zeros = [
        np.zeros((N_CORES * a.shape[0], *a.shape[1:]), a.dtype)
        for a in r["out_avals"]
    ]
    return [jax.device_put(x) for x in concat_in + concat_zeros]


def _execute(in_maps, n_timing_iters=0):
    """Run the compiled program with per-core inputs. Returns (results, times)."""
    import time
    import jax

    r = _get_runner()
    args = _stage_args(r, in_maps)
    out = r["sharded"](*args)
    jax.block_until_ready(out)

    times = []
    for _ in range(n_timing_iters):
        t0 = time.perf_counter()
        out2 = r["sharded"](*args)
        jax.block_until_ready(out2)
        times.append(time.perf_counter() - t0)

    results = []
    for c in range(N_CORES):
        d = {}
        for i, name in enumerate(r["out_names"]):
            shp = r["out_avals"][i].shape
            d[name] = np.asarray(out[i]).reshape(N_CORES, *shp)[c]
        results.append(d)
    return results, times


def _core_pairs(c):
    """(z, g) pairs owned by core c."""
    return [((p // G), (p % G)) for p in (2 * c, 2 * c + 1)]


def _prepare_in_maps(q, k_blocks, v_blocks, block_ends):
    q = np.asarray(q, dtype=np.float32)
    k_blocks = np.asarray(k_blocks, dtype=np.float32)
    v_blocks = np.asarray(v_blocks, dtype=np.float32)
    block_ends = np.asarray(block_ends)

    qt_all = np.ascontiguousarray(q.transpose(0, 1, 3, 2))  # [Z,H,128,4096]
    maskT = (block_ends[:, None] <= np.arange(S)[None, :]).astype(np.uint8)

    in_maps = []
    for c in range(N_CORES):
        qT = np.empty((128, Q_PER_CORE), np.float32)
        kT = np.empty((PAIRS_PER_CORE, 128, 128), np.float32)
        v = np.empty((PAIRS_PER_CORE, 128, 128), np.float32)
        for pi, (z, g) in enumerate(_core_pairs(c)):
            kT[pi] = k_blocks[z, g].T
            v[pi] = v_blocks[z, g]
            for j in range(HPG):
                seg = (pi * HPG + j) * S
                qT[:, seg:seg + S] = qt_all[z, g * HPG + j]
        in_maps.append({"qT": qT, "kT": kT, "v": v, "maskT": maskT,
                        "ones": np.ones((128, 1), np.float32)})
    return in_maps


def _assemble_output(results):
    out = np.empty((Z, H, S, DV), np.float32)
    for c in range(N_CORES):
        oT = results[c]["oT"]
        l = results[c]["l"][0]
        for pi, (z, g) in enumerate(_core_pairs(c)):
            for j in range(HPG):
                seg = (pi * HPG + j) * S
                lj = l[seg:seg + S]
                oj = oT[:, seg:seg + S]
                denom = np.where(lj > 0, lj, 1.0)
                out[z, g * HPG + j] = np.where(
                    lj[None, :] > 0, oj / denom[None, :], 0.0).T
    return out


def kernel(q, k_blocks, v_blocks, block_ends):
    in_maps = _prepare_in_maps(q, k_blocks, v_blocks, block_ends)
    results, _ = _execute(in_maps)
    return _assemble_output(results)


def kernel_timed(q, k_blocks, v_blocks, block_ends, n_iters=30):
    """Like kernel() but also returns per-call device wall times (seconds)."""
    in_maps = _prepare_in_maps(q, k_blocks, v_blocks, block_ends)
    results, times = _execute(in_maps, n_timing_iters=n_iters)
    return _assemble_output(results), times


def measure_exec_ns(q, k_blocks, v_blocks, block_ends, k_long=None, n_reps=10):
    """Estimate per-execution device time: run a program variant whose body is
    repeated k_long times inside the NEFF, difference against the 1x program
    (removes the ~100ms axon per-dispatch overhead)."""
    import time
    import jax

    if k_long is None:
        k_long = int(os.environ.get("NSA_TIMING_REPS", "9"))
    in_maps = _prepare_in_maps(q, k_blocks, v_blocks, block_ends)

    def best_time(r, nrep):
        args = _stage_args(r, in_maps)
        fn = r["sharded"]
        out = fn(*args)
        jax.block_until_ready(out)  # warm: compile + first exec
        best = float("inf")
        for _ in range(nrep):
            t0 = time.perf_counter()
            out = fn(*args)
            jax.block_until_ready(out)
            best = min(best, time.perf_counter() - t0)
        return best

    t1 = best_time(_get_runner(1), n_reps)
    tk = best_time(_get_runner(k_long), n_reps)
    per_exec_s = (tk - t1) / (k_long - 1)
    return per_exec_s * 1e9, t1, tk
